# revision 51
# baseline (speedup 1.0000x reference)
"""Trainium2 Bass kernel for the AgeSAGE GNN problem (8 NeuronCores).

Self-contained: builds the SPMD Bass program at call time from the edge
list, shards inputs across 8 cores, runs via run_bass_kernel_spmd, and
returns the full [256] output.

Strategy:
- Nodes dst-partitioned across 8 cores, dst-windows of 125 nodes.
- Edges bucketed per (dst-window j, src-half h); buckets padded to x128
  chunks, uniform across cores (max over cores) so the SPMD instruction
  stream is identical on all cores.
- Per edge: dma_gather (4 SWDGE queues, 64-bf16 elements out of
  256B-stride 4-node-packed tables) pulls the half-row holding the source
  node; a parity-coded indicator [128,250] (DVE is_equal vs iota) and two
  PE matmuls (even/odd 32-col halves) accumulate the segment sum into a
  per-window PSUM region [M,125].
- Per-node state lives partition-packed: window w at partition base
  32*(w%PACK), free columns (w//PACK)*125.. so SBUF is used densely.
- BatchNorm via second-moment matrices C = sum_w [U_w|1]^T [U_w|1] (PE),
  AllReduce, then per-feature affine+ReLU fused into one ACT op/window.
- Layer handoff: local h1 shard cast to bf16, AllGathered into the packed
  gather table.
- Graph mean-pool via PE matmul with a host-built 1/cnt-folded indicator,
  AllReduce, small head.
"""

import contextlib

import numpy as np
import ml_dtypes

import concourse.bass as bass
import concourse.bacc as bacc
import concourse.mybir as mybir
import concourse.tile as tile
from bass_rust import DependencyInfo
from concourse import ap_utils
from concourse.library_config import mlp as mlp_lib
from concourse._compat import round_up_to_multiple

FP = mybir.dt.float32
BF = mybir.dt.bfloat16
I16 = mybir.dt.int16

N_CORES = 8
WIN = 125
EPS = 1e-5
CALL_MAX = 1024
NQ = 4


def exact_div(a, b):
    assert a % b == 0, (a, b)
    return a // b


def dma_gather_raw(gp, out_ap, in_ap, idxs_ap, num_idxs, elem_size, elem_step,
                   queue_num=0):
    """bass.dma_gather without the elem_size%256 assert (the 256B constraint
    is on the row stride; elem_size is packet-based)."""
    assert idxs_ap.dtype == I16
    assert in_ap.dtype == out_ap.dtype
    assert ap_utils.ap_is_contiguous(in_ap.ap[1:])
    assert ap_utils.ap_is_contiguous(out_ap.ap[1:])
    assert ap_utils.ap_is_contiguous(idxs_ap.ap[1:])
    assert in_ap.ap[-1][1] == out_ap.ap[-1][1] == elem_size
    assert out_ap.ap[0][1] * out_ap.ap[1][1] == round_up_to_multiple(num_idxs, 128)
    assert in_ap.ap[0][0] == elem_step
    stride_bytes = elem_step * mybir.dt.size(in_ap.dtype)
    stride_bytes_256 = exact_div(stride_bytes, 256)
    assert stride_bytes_256 < 256
    _in_ap = gp.lower_ap_dma(in_ap, for_custom_bir_dma=True)
    _idxs_ap = gp.lower_ap(idxs_ap)
    _out_ap = gp.lower_ap(out_ap)
    return gp.add_instruction(
        mybir.InstDMAGatherAnt(
            name=gp.bass.get_next_instruction_name(),
            ins=[*_in_ap, _idxs_ap, gp.lower_val_access(gp.to_reg(num_idxs))],
            outs=[_out_ap],
            transpose=False,
            num_idxs=num_idxs,
            elem_size=elem_size,
            stride_bytes_256=stride_bytes_256,
            gen_mode=0,
            single_packet=True,
            queue_num=queue_num,
            sbuf_tokens_per_rank=0,
            sbuf_free_dim_per_rank=0,
            sbuf_free_dim_pad_per_rank=0,
            sbuf_byte_offset=0,
        )
    )


# ----------------------------------------------------------------------------
# Host-side schedule / input prep (index math + layout only)
# ----------------------------------------------------------------------------

class Schedule:
    pass


def build_schedule(x, edge_index, batch, n_nodes, n_graphs, wgroup=2):
    s = Schedule()
    assert n_nodes % (N_CORES * WIN) == 0
    npc = n_nodes // N_CORES
    n_win = npc // WIN
    s.n_nodes, s.n_graphs = n_nodes, n_graphs
    s.nodes_per_core, s.n_win = npc, n_win
    s.wgroup = wgroup
    assert n_win % wgroup == 0
    s.n_wg = n_win // wgroup
    s.pack = 4 if n_win % 4 == 0 else (2 if n_win % 2 == 0 else 1)
    s.freep = round_up_to_multiple((n_win // s.pack) * WIN, 16)

    src = np.asarray(edge_index[0], np.int64)
    dst = np.asarray(edge_index[1], np.int64)
    E = src.shape[0]

    deg = np.bincount(dst, minlength=n_nodes).astype(np.float32)
    recip_deg = (1.0 / np.maximum(deg, 1.0)).astype(np.float32)

    core = dst // npc
    j = (dst % npc) // WIN
    h = (src >> 1) & 1
    key = (core * n_win + j) * 2 + h
    order = np.argsort(key, kind="stable")
    src_s, dst_s, key_s = src[order], dst[order], key[order]
    counts = np.bincount(key_s, minlength=N_CORES * n_win * 2).reshape(
        N_CORES, n_win, 2)
    L = np.ceil(counts.max(axis=0) / 128).astype(np.int64) * 128
    L = np.maximum(L, 128)
    s.L = L
    s.pad_frac = float(L.sum() * N_CORES) / max(E, 1) - 1.0

    # h-major slot order: all h=0 buckets first, then all h=1. Gather calls
    # tile each h-phase densely (all full CALL_MAX except one per phase);
    # per-window aggregation is split into two PSUM accumulations with the
    # h=0 partial parked in SBUF.
    slot_runs = []
    for hh in range(2):
        for wg in range(s.n_wg):
            for jj in range(wg * wgroup, (wg + 1) * wgroup):
                slot_runs.append((wg, hh, jj, int(L[jj, hh])))
    s.slot_runs = slot_runs
    s.nslot = int(L.sum())
    s.nchunk = s.nslot // 128
    s.nchunk8 = round_up_to_multiple(s.nchunk, 8)

    chunk_meta = []
    for (wg, hh, jj, ln) in slot_runs:
        nck = ln // 128
        for c in range(nck):
            chunk_meta.append((jj, hh, c == 0, c == nck - 1))
    s.chunk_meta = chunk_meta

    calls = []
    off = 0
    for hh in range(2):
        run_len = int(L[:, hh].sum())
        ro = 0
        while ro < run_len:
            n = min(CALL_MAX, run_len - ro)
            calls.append((off + ro, n, hh))
            ro += n
        off += run_len
    assert off == s.nslot
    s.calls = calls

    gcnt = np.bincount(batch, minlength=n_graphs).astype(np.float32)
    recip_gcnt = (1.0 / np.maximum(gcnt, 1.0)).astype(np.float32)

    per_core = []
    starts = np.zeros(N_CORES * n_win * 2 + 1, np.int64)
    np.cumsum(np.bincount(key_s, minlength=N_CORES * n_win * 2), out=starts[1:])

    x = np.asarray(x, np.float32)
    n_pack_rows = n_nodes // 4
    xtab = np.zeros((n_pack_rows, 128), ml_dtypes.bfloat16)
    xv = x.astype(ml_dtypes.bfloat16)
    for q in range(4):
        xtab[:, 32 * q:32 * q + 3] = xv[q::4, :]

    PACK = s.pack
    for c in range(N_CORES):
        idx_slots = np.zeros(s.nslot, np.int16)
        pcode = np.full(s.nslot, 384.0, np.float32)
        off = 0
        for (wg, hh, jj, ln) in slot_runs:
            b = (c * n_win + jj) * 2 + hh
            e0, e1 = starts[b], starts[b + 1]
            cnt = e1 - e0
            assert cnt <= ln
            esrc, edst = src_s[e0:e1], dst_s[e0:e1]
            idx_slots[off:off + cnt] = (esrc >> 2).astype(np.int16)
            pcode[off:off + cnt] = (edst - (c * npc + jj * WIN)) \
                + WIN * (esrc & 1)
            off += ln
        idx16 = np.zeros((128, s.nslot // 16), np.int16)
        for (so, n, hh) in calls:
            blk = idx_slots[so:so + round_up_to_multiple(n, 16)]
            blk = blk.reshape(-1, 16).T
            idx16[:, so // 16: so // 16 + blk.shape[1]] = np.tile(blk, (8, 1))
        pc = np.full((128, s.nchunk8), 384.0, np.float32)
        pc[:, :s.nchunk] = pcode.reshape(s.nchunk, 128).T
        pc = pc.astype(ml_dtypes.bfloat16)

        nodes0 = c * npc
        x_loc = x[nodes0:nodes0 + npc]
        rd_loc = recip_deg[nodes0:nodes0 + npc]

        xp = np.zeros((128, s.freep), np.float32)
        rdp = np.zeros((128, s.freep), np.float32)
        for w in range(n_win):
            bp = 32 * (w % PACK)
            c0 = (w // PACK) * WIN
            xp[bp:bp + 3, c0:c0 + WIN] = x_loc[w * WIN:(w + 1) * WIN].T
            rdp[bp:bp + 32, c0:c0 + WIN] = rd_loc[w * WIN:(w + 1) * WIN]

        pool_ind = np.zeros((npc, 256), ml_dtypes.bfloat16)
        bloc = np.asarray(batch[nodes0:nodes0 + npc], np.int64)
        pf = np.zeros((npc, 256), np.float32)
        pf[np.arange(npc), bloc] = recip_gcnt[bloc]
        pool_ind[:] = pf.astype(ml_dtypes.bfloat16)

        per_core.append(dict(
            idx16=idx16, pcode=pc, xp=xp, rdp=rdp,
            pool_ind=pool_ind, xtab=xtab,
        ))
    return s, per_core


def build_weight_inputs(s, w1l, b1l, w1r, bn1_g, bn1_b, w2l, b2l, w2r,
                        bn2_g, bn2_b, wlin, blin):
    PACK = s.pack

    def rep_rows(w, k):
        out = np.zeros((128, 32), np.float32)
        for g in range(PACK):
            out[32 * g:32 * g + k] = np.asarray(w, np.float32)
        return out

    col = lambda v: np.asarray(v, np.float32).reshape(-1, 1)
    iota = np.tile(np.arange(250, dtype=np.float32), 8).reshape(1, 2000)
    iota = np.repeat(iota, 128, 0).astype(ml_dtypes.bfloat16)
    return dict(
        W1l=rep_rows(w1l, 3), W1r=rep_rows(w1r, 3),
        W2l=rep_rows(w2l, 32), W2r=rep_rows(w2r, 32),
        fold=np.tile(np.eye(32, dtype=np.float32), (4, 1)),
        b1l=col(b1l), bn1_g=col(bn1_g), bn1_b=col(bn1_b),
        b2l=col(b2l), bn2_g=col(bn2_g), bn2_b=col(bn2_b),
        wlin=col(wlin),
        blin_rep=np.full((128, 1), float(np.asarray(blin).ravel()[0]),
                         np.float32),
        iden=np.eye(128, dtype=np.float32),
        idenb=np.eye(128, dtype=ml_dtypes.bfloat16),
        iota=iota,
    )


# ----------------------------------------------------------------------------
# Device program
# ----------------------------------------------------------------------------

def build_program(s):
    n_pack_rows = s.n_nodes // 4
    npc = s.nodes_per_core

    nc = bacc.Bacc("TRN2", target_bir_lowering=False, debug=False,
                   num_devices=N_CORES, num_swdge_queues=NQ)

    def din(name, shape, dt):
        return nc.dram_tensor(name, shape, dt, kind="ExternalInput").ap()

    T = dict(
        xtab=din("xtab", [n_pack_rows, 128], BF),
        idx16=din("idx16", [128, s.nslot // 16], I16),
        pcode=din("pcode", [128, s.nchunk8], BF),
        xp=din("xp", [128, s.freep], FP),
        rdp=din("rdp", [128, s.freep], FP),
        pool=din("pool_ind", [npc, 256], BF),
        W1l=din("W1l", [128, 32], FP),
        W1r=din("W1r", [128, 32], FP),
        W2l=din("W2l", [128, 32], FP),
        W2r=din("W2r", [128, 32], FP),
        fold=din("fold", [128, 32], FP),
        b1l=din("b1l", [32, 1], FP),
        bn1g=din("bn1_g", [32, 1], FP),
        bn1b=din("bn1_b", [32, 1], FP),
        b2l=din("b2l", [32, 1], FP),
        bn2g=din("bn2_g", [32, 1], FP),
        bn2b=din("bn2_b", [32, 1], FP),
        wlin=din("wlin", [32, 1], FP),
        blin=din("blin_rep", [128, 1], FP),
        iden=din("iden", [128, 128], FP),
        idenb=din("idenb", [128, 128], BF),
        iota=din("iota", [128, 2000], BF),
        out=nc.dram_tensor("out", [256], FP, kind="ExternalOutput").ap(),
        h1loc=nc.dram_tensor("h1loc", [npc * 32], BF).ap(),
        htab=nc.dram_tensor("htab", [n_pack_rows, 128], BF).ap(),
    )

    with tile.TileContext(nc) as tc:
        _body(tc, nc, s, T)
    nc.compile()
    return nc


def _body(tc, nc, s, T):
    npc = s.nodes_per_core
    n_win = s.n_win
    PACK = s.pack
    RG = [list(range(N_CORES))]
    AOT = mybir.AluOpType
    AFT = mybir.ActivationFunctionType

    stack = contextlib.ExitStack()
    pl = {}
    pl["P"] = stack.enter_context(tc.tile_pool(name="persist", bufs=1))
    pl["ring"] = stack.enter_context(tc.tile_pool(name="ring", bufs=16))
    pl["ind"] = stack.enter_context(tc.tile_pool(name="ind", bufs=5))
    pl["pcx"] = stack.enter_context(tc.tile_pool(name="pcx", bufs=2))
    pl["win"] = stack.enter_context(tc.tile_pool(name="win", bufs=3))
    pl["mw"] = stack.enter_context(tc.tile_pool(name="mw", bufs=3))
    pl["agg"] = stack.enter_context(tc.tile_pool(name="agg", bufs=4, space="PSUM"))
    pl["tp"] = stack.enter_context(tc.tile_pool(name="tp", bufs=3, space="PSUM"))
    pl["cp"] = stack.enter_context(tc.tile_pool(name="cp", bufs=1, space="PSUM"))
    pl["dram"] = stack.enter_context(tc.tile_pool(name="dram", bufs=1, space="DRAM"))
    pl["sm"] = stack.enter_context(tc.tile_pool(name="small", bufs=1))
    P = pl["P"]

    lib_inst = nc.gpsimd.load_library(mlp_lib)
    gst = {"count": 0, "prev": lib_inst}

    def load(name, shape, dt, src):
        t = P.tile(shape, dt, tag=name, name=name)
        nc.sync.dma_start(out=t[:], in_=src)
        return t

    # idx split: first gather call's slice loads alone so the layer-1 gather
    # stream starts without waiting for the full 6.5MB index table.
    nA = min(64, s.nslot // 16)
    idxA_sb = load("idxA", [128, nA], I16, T["idx16"][:, 0:nA])
    idxB_sb = load("idxB", [128, s.nslot // 16 - nA], I16, T["idx16"][:, nA:])

    def idx_slice(so, n):
        c0 = so // 16
        c1 = c0 + round_up_to_multiple(n, 16) // 16
        if c1 <= nA:
            return idxA_sb[:, c0:c1]
        assert c0 >= nA, (so, n)
        return idxB_sb[:, c0 - nA:c1 - nA]

    pcode_sb = load("pcodes", [128, s.nchunk8], BF, T["pcode"])
    iota_sb = load("iotas", [128, 2000], BF, T["iota"])
    rdp_sb = load("rdps", [128, s.freep], FP, T["rdp"])
    xp_sb = load("xps", [128, s.freep], FP, T["xp"])
    W1l_sb = load("W1ls", [128, 32], FP, T["W1l"])
    W1r_sb = load("W1rs", [128, 32], FP, T["W1r"])
    W2l_sb = load("W2ls", [128, 32], FP, T["W2l"])
    W2r_sb = load("W2rs", [128, 32], FP, T["W2r"])
    iden_sb = load("idens", [128, 128], FP, T["iden"])
    fold_sb = load("folds", [128, 32], FP, T["fold"])
    idenb_sb = load("idenbs", [128, 128], BF, T["idenb"])
    b1l_sb = load("b1ls", [32, 1], FP, T["b1l"])
    bn1g_sb = load("bn1gs", [32, 1], FP, T["bn1g"])
    bn1b_sb = load("bn1bs", [32, 1], FP, T["bn1b"])
    b2l_sb = load("b2ls", [32, 1], FP, T["b2l"])
    bn2g_sb = load("bn2gs", [32, 1], FP, T["bn2g"])
    bn2b_sb = load("bn2bs", [32, 1], FP, T["bn2b"])
    wlin_sb = load("wlins", [32, 1], FP, T["wlin"])
    blin_sb = load("blins", [128, 1], FP, T["blin"])

    aggsave = P.tile([128, s.freep], FP, tag="aggsave", name="aggsave")
    h1p = P.tile([128, s.freep], FP, tag="h1p", name="h1p")
    h1preTp = P.tile([128, s.freep], FP, tag="h1preTp", name="h1preTp")
    h2preTp = P.tile([128, s.freep], FP, tag="h2preTp", name="h2preTp")
    h2Tp = P.tile([128, s.freep], BF, tag="h2Tp", name="h2Tp")
    h1pack = P.tile([128, n_win * 32], BF, tag="h1pack", name="h1pack")

    def wpos(w):
        return (32 * (w % PACK),
                slice((w // PACK) * WIN, (w // PACK) * WIN + WIN))

    # ------------------------------------------------------------------
    def run_layer(layer, table_ap, Mrows):
        layer_state = {}
        # per-window BN stat slots: window w = PACK*k+g writes
        # [32g:32g+32, k] — filled by ACT accum_out on the preT copy (sum)
        # and a Square activation (sum of squares).
        msumb = pl["sm"].tile([128, n_win // PACK], FP,
                              tag=f"msumb{layer}", name="msumb")
        msqb = pl["sm"].tile([128, n_win // PACK], FP,
                             tag=f"msqb{layer}", name="msqb")
        win_psum = {}
        chunk_i = 0
        for (so, n, hh) in s.calls:
            ring_t = pl["ring"].tile([128, (CALL_MAX // 128) * 64], BF,
                                     tag="ring", name="ringt")
            gi = dma_gather_raw(
                nc.gpsimd,
                ring_t[:, 0:(round_up_to_multiple(n, 128) // 128) * 64]
                    .rearrange("p (s e) -> p s e", e=64),
                table_ap[:, 64 * hh:64 * hh + 64],
                idx_slice(so, n),
                n, 64, 128,
                queue_num=gst["count"] % NQ,
            )
            gi.ins.add_dependency(gst["prev"].ins.name,
                                  DependencyInfo.NO_SYNC_ONLY)
            gst["count"] += 1
            gst["prev"] = gi
            ncks = round_up_to_multiple(n, 128) // 128
            for ck in range(ncks):
                jj, hh2, first, last = s.chunk_meta[chunk_i]
                bp, wcol = wpos(jj)
                if chunk_i % 8 == 0:
                    ind8 = pl["ind"].tile([128, 2000], BF, tag="ind",
                                          name="ind8t")
                    g0 = chunk_i
                    if (chunk_i // 8) % 4 != 3:
                        # expand pcode on the (idle) scalar engine so the
                        # is_equal has all-packed operands → 2x DVE mode
                        pcx = pl["pcx"].tile([128, 2000], BF, tag="pcx",
                                             name="pcxt")
                        nc.scalar.activation(
                            pcx[:].rearrange("p (c o) -> p c o", o=250),
                            pcode_sb[:, g0:g0 + 8].to_broadcast([128, 8, 250]),
                            AFT.Copy)
                        nc.vector.tensor_tensor(
                            ind8[:], iota_sb[:, 0:2000], pcx[:], AOT.is_equal)
                    else:
                        nc.vector.tensor_tensor(
                            ind8[:].rearrange("p (c o) -> p c o", o=250),
                            iota_sb[:, 0:2000]
                                .rearrange("p (c o) -> p c o", o=250),
                            pcode_sb[:, g0:g0 + 8].to_broadcast([128, 8, 250]),
                            AOT.is_equal)
                    layer_state["ind8"] = ind8
                ind8 = layer_state["ind8"]
                co = (chunk_i % 8) * 250
                if jj not in win_psum:
                    win_psum[jj] = pl["agg"].tile(
                        [128, 128], FP, space="PSUM", tag="agg", name="aggt")
                ps = win_psum[jj]
                nc.tensor.matmul(
                    ps[bp:bp + Mrows, 0:125],
                    lhsT=ring_t[:, ck * 64: ck * 64 + Mrows],
                    rhs=ind8[:, co:co + 125],
                    start=first, stop=False,
                    tile_position=(0, bp))
                nc.tensor.matmul(
                    ps[bp:bp + Mrows, 0:125],
                    lhsT=ring_t[:, ck * 64 + 32: ck * 64 + 32 + Mrows],
                    rhs=ind8[:, co + 125:co + 250],
                    start=False, stop=last,
                    tile_position=(0, bp))
                chunk_i += 1
                if last and hh == 0:
                    # park the h=0 partial in SBUF; phase 1 adds it back
                    ps_t = win_psum.pop(jj)
                    nc.scalar.activation(aggsave[bp:bp + Mrows, wcol],
                                         ps_t[bp:bp + Mrows, 0:125], AFT.Copy)
                if last and hh == 1:
                    ps_t = win_psum.pop(jj)
                    meanw = pl["mw"].tile([128, 128], FP, tag="mw", name="mwt")
                    nc.vector.tensor_tensor(
                        meanw[bp:bp + Mrows, 0:125], ps_t[bp:bp + Mrows, 0:125],
                        aggsave[bp:bp + Mrows, wcol], AOT.add)
                    nc.vector.tensor_tensor(
                        meanw[bp:bp + Mrows, 0:125], meanw[bp:bp + Mrows, 0:125],
                        rdp_sb[bp:bp + Mrows, wcol], AOT.mult)
                    other = xp_sb if layer == 1 else h1p
                    ko = 3 if layer == 1 else 32
                    Wl = W1l_sb if layer == 1 else W2l_sb
                    Wr = W1r_sb if layer == 1 else W2r_sb
                    preT = h1preTp if layer == 1 else h2preTp
                    kcol = jj // PACK
                    pw = pl["tp"].tile([128, 128], FP, space="PSUM",
                                       tag="tp", name="pwt")
                    nc.tensor.matmul(
                        pw[bp:bp + 32, 0:125], lhsT=Wl[bp:bp + Mrows, :],
                        rhs=meanw[bp:bp + Mrows, 0:125], start=True, stop=False,
                        tile_position=(bp, bp))
                    nc.tensor.matmul(
                        pw[bp:bp + 32, 0:125], lhsT=Wr[bp:bp + ko, :],
                        rhs=other[bp:bp + ko, wcol], start=False, stop=True,
                        tile_position=(bp, bp))
                    nc.scalar.activation(preT[bp:bp + 32, wcol],
                                         pw[bp:bp + 32, 0:125], AFT.Copy,
                                         accum_out=msumb[bp:bp + 32,
                                                         kcol:kcol + 1])
                    sqd = pl["win"].tile([128, 128], FP, tag="sqd", name="sqd")
                    nc.scalar.activation(sqd[bp:bp + 32, 0:125],
                                         pw[bp:bp + 32, 0:125], AFT.Square,
                                         accum_out=msqb[bp:bp + 32,
                                                        kcol:kcol + 1])
        assert chunk_i == s.nchunk
        # reduce window slots and fold the 4 pack groups via one PE matmul
        statspair = pl["sm"].tile([128, 2], FP, tag=f"sp{layer}", name="spt")
        nc.vector.tensor_reduce(statspair[:, 0:1], msqb[:, :],
                                mybir.AxisListType.X, AOT.add)
        nc.vector.tensor_reduce(statspair[:, 1:2], msumb[:, :],
                                mybir.AxisListType.X, AOT.add)
        proj_ps = pl["tp"].tile([32, 2], FP, space="PSUM", tag="tp",
                                name="projps")
        nc.tensor.matmul(proj_ps[:, :], lhsT=fold_sb[:, :], rhs=statspair[:, :],
                         start=True, stop=True)
        proj = pl["sm"].tile([32, 2], FP, tag=f"proj{layer}", name="projt")
        nc.vector.tensor_copy(proj[:, :], proj_ps[:, :])
        return proj

    def allreduce_sb(src_sb_ap, shape, tag):
        dti = pl["dram"].tile(shape, FP, tag=f"ari{tag}", name="arit")
        dto = pl["dram"].tile(shape, FP, tag=f"aro{tag}", name="arot")
        nc.sync.dma_start(out=dti[:], in_=src_sb_ap)
        nc.gpsimd.collective_compute(
            "AllReduce", AOT.add, replica_groups=RG,
            ins=[dti.opt()], outs=[dto.opt()])
        res = pl["sm"].tile(shape, FP, tag=f"arr{tag}", name="arrt")
        nc.sync.dma_start(out=res[:], in_=dto[:])
        return res

    def bn_post(pg, bl_sb, g_sb, b_sb, tag):
        # var*N = pg0 - Ninv*pg1^2 ; std = sqrt(Ninv*(var*N) + EPS)
        SM = pl["sm"]
        Ninv = 1.0 / s.n_nodes
        t1 = SM.tile([32, 1], FP, tag=f"t1{tag}", name="t1s")
        nc.vector.tensor_tensor(t1[:], pg[:, 1:2], pg[:, 1:2], AOT.mult)
        nc.vector.tensor_scalar(t1[:], t1[:], -Ninv, None, AOT.mult)
        nc.vector.tensor_tensor(t1[:], t1[:], pg[:, 0:1], AOT.add)
        epsc = SM.tile([32, 1], FP, tag=f"eps{tag}", name="epsc")
        nc.vector.memset(epsc[:], EPS)
        sq = SM.tile([32, 1], FP, tag=f"sq{tag}", name="sqt")
        nc.scalar.activation(sq[:], t1[:], AFT.Sqrt, bias=epsc[:], scale=Ninv)
        y = SM.tile([32, 1], FP, tag=f"y{tag}", name="yt")
        nc.vector.reciprocal(y[:], sq[:])
        a = SM.tile([128, 1], FP, tag=f"a{tag}", name="at")
        cc = SM.tile([128, 1], FP, tag=f"c{tag}", name="cct")
        nc.vector.tensor_tensor(a[0:32], g_sb[:], y[:], AOT.mult)
        t = SM.tile([32, 1], FP, tag=f"t{tag}", name="tt")
        nc.vector.tensor_scalar(t[:], pg[:, 1:2], Ninv, None, AOT.mult)
        nc.vector.tensor_tensor(t[:], t[:], bl_sb[:], AOT.add)
        nc.vector.tensor_tensor(t[:], t[:], a[0:32], AOT.mult)
        nc.vector.tensor_tensor(cc[0:32], b_sb[:], t[:], AOT.subtract)
        for g in range(1, PACK):
            nc.sync.dma_start(out=a[32 * g:32 * g + 32], in_=a[0:32])
            nc.sync.dma_start(out=cc[32 * g:32 * g + 32], in_=cc[0:32])
        return a, cc

    # ================= LAYER 1 =================
    p1 = run_layer(1, T["xtab"], 3)
    p1g = allreduce_sb(p1[:], [32, 2], "c1")
    a1, c1 = bn_post(p1g, b1l_sb, bn1g_sb, bn1b_sb, "1")

    # finalize batched per pack-group: windows 4k..4k+3 fill all 128
    # partitions of column block k, and their h1pack column blocks are
    # contiguous, so one relu + one full-height transpose covers 4 windows.
    assert PACK == 4 and n_win % PACK == 0
    for k in range(n_win // PACK):
        wcol = slice(k * WIN, (k + 1) * WIN)
        nc.scalar.activation(h1p[:, wcol], h1preTp[:, wcol],
                             AFT.Relu, bias=c1[:], scale=a1[:])
    for k in range(n_win // PACK):
        wcol = slice(k * WIN, (k + 1) * WIN)
        tps = pl["tp"].tile([128, 128], FP, space="PSUM", tag="tp", name="h1tp")
        nc.tensor.transpose(tps[0:125, 0:128], h1p[:, wcol], iden_sb[:, :],
                            tile_position=(0, 0))
        nc.scalar.activation(h1pack[0:125, k * 128:(k + 1) * 128],
                             tps[0:125, 0:128], AFT.Copy)
    nc.sync.dma_start(
        out=T["h1loc"].rearrange("(w p f) -> p w f", p=WIN, f=32),
        in_=h1pack[0:125, 0:n_win * 32].rearrange("p (w f) -> p w f", f=32))
    nc.gpsimd.collective_compute(
        "AllGather", AOT.bypass, replica_groups=RG,
        ins=[T["h1loc"][:]], outs=[T["htab"].rearrange("a b -> (a b)")])

    # ================= LAYER 2 =================
    p2 = run_layer(2, T["htab"], 32)
    p2g = allreduce_sb(p2[:], [32, 2], "c2")
    a2, c2 = bn_post(p2g, b2l_sb, bn2g_sb, bn2b_sb, "2")

    pool_ps = pl["cp"].tile([32, 256], FP, space="PSUM", tag="C",
                            name="poolps")
    for k in range(n_win // PACK):
        wcol = slice(k * WIN, (k + 1) * WIN)
        nc.scalar.activation(h2Tp[:, wcol], h2preTp[:, wcol],
                             AFT.Relu, bias=c2[:], scale=a2[:])
        tps = pl["tp"].tile([128, 128], BF, space="PSUM", tag="tp", name="h2tp")
        nc.tensor.transpose(tps[0:125, 0:128], h2Tp[:, wcol], idenb_sb[:, :],
                            tile_position=(0, 0))
        h2w = pl["win"].tile([128, 128], BF, tag="h2w", name="h2w")
        nc.scalar.activation(h2w[0:125, :], tps[0:125, 0:128], AFT.Copy)
        pind4 = pl["win"].tile([128, PACK * 256], BF, tag="pind4", name="pind4")
        nc.sync.dma_start(
            out=pind4[0:125, :].rearrange("p (g c) -> p g c", c=256),
            in_=T["pool"][k * PACK * WIN:(k + 1) * PACK * WIN, :]
                .rearrange("(g n) c -> n g c", g=PACK))
        for g in range(PACK):
            w = PACK * k + g
            nc.tensor.matmul(pool_ps[:, :], lhsT=h2w[0:125, 32 * g:32 * g + 32],
                             rhs=pind4[0:125, 256 * g:256 * g + 256],
                             start=w == 0, stop=w == n_win - 1)

    gsum_sb = pl["sm"].tile([32, 256], FP, tag="gsum", name="gsum_sb")
    nc.vector.tensor_copy(gsum_sb[:], pool_ps[:, :])
    gT = allreduce_sb(gsum_sb[:], [32, 256], "pool")

    gw = pl["sm"].tile([32, 256], FP, tag="gw", name="gw")
    nc.vector.tensor_scalar(gw[:], gT[:], wlin_sb[:], None, AOT.mult)
    ones32 = pl["sm"].tile([32, 1], FP, tag="ones32", name="ones32")
    nc.vector.memset(ones32[:], 1.0)
    ostage = pl["sm"].tile([128, 2], FP, tag="ostage", name="ostage")
    for half in range(2):
        hp = pl["tp"].tile([128, 1], FP, space="PSUM", tag="tp", name="hpt")
        nc.tensor.matmul(hp[:, :], lhsT=gw[:, half * 128:(half + 1) * 128],
                         rhs=ones32[:, :], start=True, stop=True)
        nc.vector.tensor_tensor(ostage[:, half:half + 1], hp[:, :],
                                blin_sb[:], AOT.add)
    nc.sync.dma_start(out=T["out"][0:128], in_=ostage[:, 0:1])
    nc.sync.dma_start(out=T["out"][128:256], in_=ostage[:, 1:2])

    stack.close()


# ----------------------------------------------------------------------------
# Public entry point
# ----------------------------------------------------------------------------

N_NODES = 100000
N_GRAPHS = 256

LAST_EXEC_NS = None


def kernel(x, edge_index, batch, w1l, b1l, w1r, bn1_g, bn1_b,
           w2l, b2l, w2r, bn2_g, bn2_b, wlin, blin, _trace=False):
    global LAST_EXEC_NS
    from concourse.bass_utils import run_bass_kernel_spmd

    x = np.asarray(x, np.float32)
    edge_index = np.asarray(edge_index)
    batch = np.asarray(batch)

    s, per_core = build_schedule(x, edge_index, batch, N_NODES, N_GRAPHS,
                                 wgroup=4)
    wts = build_weight_inputs(s, w1l, b1l, w1r, bn1_g, bn1_b,
                              w2l, b2l, w2r, bn2_g, bn2_b, wlin, blin)
    nc = build_program(s)
    in_maps = []
    for c in range(N_CORES):
        m = dict(per_core[c])
        m.update(wts)
        in_maps.append(m)
    res = run_bass_kernel_spmd(nc, in_maps, list(range(N_CORES)),
                               trace=_trace)
    LAST_EXEC_NS = res.exec_time_ns
    return np.asarray(res.results[0]["out"], np.float32)



# revision 52
# speedup vs baseline: 1.1325x; 1.1325x over previous
"""Trainium2 Bass kernel for the AgeSAGE GNN problem (8 NeuronCores).

Self-contained: builds the SPMD Bass program at call time from the edge
list, shards inputs across 8 cores, runs via run_bass_kernel_spmd, and
returns the full [256] output.

Strategy:
- Nodes dst-partitioned across 8 cores, dst-windows of 125 nodes.
- Edges bucketed per (dst-window j, src-half h); buckets padded to x128
  chunks, uniform across cores (max over cores) so the SPMD instruction
  stream is identical on all cores.
- Per edge: dma_gather (4 SWDGE queues, 64-bf16 elements out of
  256B-stride 4-node-packed tables) pulls the half-row holding the source
  node; a parity-coded indicator [128,250] (DVE is_equal vs iota) and two
  PE matmuls (even/odd 32-col halves) accumulate the segment sum into a
  per-window PSUM region [M,125].
- Per-node state lives partition-packed: window w at partition base
  32*(w%PACK), free columns (w//PACK)*125.. so SBUF is used densely.
- BatchNorm via second-moment matrices C = sum_w [U_w|1]^T [U_w|1] (PE),
  AllReduce, then per-feature affine+ReLU fused into one ACT op/window.
- Layer handoff: local h1 shard cast to bf16, AllGathered into the packed
  gather table.
- Graph mean-pool via PE matmul with a host-built 1/cnt-folded indicator,
  AllReduce, small head.
"""

import contextlib

import numpy as np
import ml_dtypes

import concourse.bass as bass
import concourse.bacc as bacc
import concourse.mybir as mybir
import concourse.tile as tile
from bass_rust import DependencyInfo
from concourse import ap_utils
from concourse.library_config import mlp as mlp_lib
from concourse._compat import round_up_to_multiple

FP = mybir.dt.float32
BF = mybir.dt.bfloat16
I16 = mybir.dt.int16

N_CORES = 8
WIN = 125
EPS = 1e-5
CALL_MAX = 1024
NQ = 4


def exact_div(a, b):
    assert a % b == 0, (a, b)
    return a // b


def dma_gather_raw(gp, out_ap, in_ap, idxs_ap, num_idxs, elem_size, elem_step,
                   queue_num=0):
    """bass.dma_gather without the elem_size%256 assert (the 256B constraint
    is on the row stride; elem_size is packet-based)."""
    assert idxs_ap.dtype == I16
    assert in_ap.dtype == out_ap.dtype
    assert ap_utils.ap_is_contiguous(in_ap.ap[1:])
    assert ap_utils.ap_is_contiguous(out_ap.ap[1:])
    assert ap_utils.ap_is_contiguous(idxs_ap.ap[1:])
    assert in_ap.ap[-1][1] == out_ap.ap[-1][1] == elem_size
    assert out_ap.ap[0][1] * out_ap.ap[1][1] == round_up_to_multiple(num_idxs, 128)
    assert in_ap.ap[0][0] == elem_step
    stride_bytes = elem_step * mybir.dt.size(in_ap.dtype)
    stride_bytes_256 = exact_div(stride_bytes, 256)
    assert stride_bytes_256 < 256
    _in_ap = gp.lower_ap_dma(in_ap, for_custom_bir_dma=True)
    _idxs_ap = gp.lower_ap(idxs_ap)
    _out_ap = gp.lower_ap(out_ap)
    return gp.add_instruction(
        mybir.InstDMAGatherAnt(
            name=gp.bass.get_next_instruction_name(),
            ins=[*_in_ap, _idxs_ap, gp.lower_val_access(gp.to_reg(num_idxs))],
            outs=[_out_ap],
            transpose=False,
            num_idxs=num_idxs,
            elem_size=elem_size,
            stride_bytes_256=stride_bytes_256,
            gen_mode=0,
            single_packet=True,
            queue_num=queue_num,
            sbuf_tokens_per_rank=0,
            sbuf_free_dim_per_rank=0,
            sbuf_free_dim_pad_per_rank=0,
            sbuf_byte_offset=0,
        )
    )


# ----------------------------------------------------------------------------
# Host-side schedule / input prep (index math + layout only)
# ----------------------------------------------------------------------------

class Schedule:
    pass


def build_schedule(x, edge_index, batch, n_nodes, n_graphs, wgroup=2):
    s = Schedule()
    assert n_nodes % (N_CORES * WIN) == 0
    npc = n_nodes // N_CORES
    n_win = npc // WIN
    s.n_nodes, s.n_graphs = n_nodes, n_graphs
    s.nodes_per_core, s.n_win = npc, n_win
    s.wgroup = wgroup
    assert n_win % wgroup == 0
    s.n_wg = n_win // wgroup
    s.pack = 4 if n_win % 4 == 0 else (2 if n_win % 2 == 0 else 1)
    s.freep = round_up_to_multiple((n_win // s.pack) * WIN, 16)

    src = np.asarray(edge_index[0], np.int64)
    dst = np.asarray(edge_index[1], np.int64)
    E = src.shape[0]

    deg = np.bincount(dst, minlength=n_nodes).astype(np.float32)
    recip_deg = (1.0 / np.maximum(deg, 1.0)).astype(np.float32)

    core = dst // npc
    j = (dst % npc) // WIN
    h = (src >> 1) & 1
    key = (core * n_win + j) * 2 + h
    order = np.argsort(key, kind="stable")
    src_s, dst_s, key_s = src[order], dst[order], key[order]
    counts = np.bincount(key_s, minlength=N_CORES * n_win * 2).reshape(
        N_CORES, n_win, 2)
    L = np.ceil(counts.max(axis=0) / 128).astype(np.int64) * 128
    L = np.maximum(L, 128)
    s.L = L
    s.pad_frac = float(L.sum() * N_CORES) / max(E, 1) - 1.0

    # h-major slot order: all h=0 buckets first, then all h=1. Gather calls
    # tile each h-phase densely (all full CALL_MAX except one per phase);
    # per-window aggregation is split into two PSUM accumulations with the
    # h=0 partial parked in SBUF.
    slot_runs = []
    for hh in range(2):
        for wg in range(s.n_wg):
            for jj in range(wg * wgroup, (wg + 1) * wgroup):
                slot_runs.append((wg, hh, jj, int(L[jj, hh])))
    s.slot_runs = slot_runs
    s.nslot = int(L.sum())
    s.nchunk = s.nslot // 128
    s.nchunk8 = round_up_to_multiple(s.nchunk, 8)

    chunk_meta = []
    for (wg, hh, jj, ln) in slot_runs:
        nck = ln // 128
        for c in range(nck):
            chunk_meta.append((jj, hh, c == 0, c == nck - 1))
    s.chunk_meta = chunk_meta

    calls = []
    off = 0
    for hh in range(2):
        run_len = int(L[:, hh].sum())
        ro = 0
        while ro < run_len:
            n = min(CALL_MAX, run_len - ro)
            calls.append((off + ro, n, hh))
            ro += n
        off += run_len
    assert off == s.nslot
    s.calls = calls

    gcnt = np.bincount(batch, minlength=n_graphs).astype(np.float32)
    recip_gcnt = (1.0 / np.maximum(gcnt, 1.0)).astype(np.float32)

    per_core = []
    starts = np.zeros(N_CORES * n_win * 2 + 1, np.int64)
    np.cumsum(np.bincount(key_s, minlength=N_CORES * n_win * 2), out=starts[1:])

    x = np.asarray(x, np.float32)
    n_pack_rows = n_nodes // 4
    xtab = np.zeros((n_pack_rows, 128), ml_dtypes.bfloat16)
    xv = x.astype(ml_dtypes.bfloat16)
    for q in range(4):
        xtab[:, 32 * q:32 * q + 3] = xv[q::4, :]

    PACK = s.pack
    for c in range(N_CORES):
        idx_slots = np.zeros(s.nslot, np.int16)
        pcode = np.full(s.nslot, 384.0, np.float32)
        off = 0
        for (wg, hh, jj, ln) in slot_runs:
            b = (c * n_win + jj) * 2 + hh
            e0, e1 = starts[b], starts[b + 1]
            cnt = e1 - e0
            assert cnt <= ln
            esrc, edst = src_s[e0:e1], dst_s[e0:e1]
            idx_slots[off:off + cnt] = (esrc >> 2).astype(np.int16)
            pcode[off:off + cnt] = (edst - (c * npc + jj * WIN)) \
                + WIN * (esrc & 1)
            off += ln
        idx16 = np.zeros((128, s.nslot // 16), np.int16)
        for (so, n, hh) in calls:
            blk = idx_slots[so:so + round_up_to_multiple(n, 16)]
            blk = blk.reshape(-1, 16).T
            idx16[:, so // 16: so // 16 + blk.shape[1]] = np.tile(blk, (8, 1))
        pc = np.full((128, s.nchunk8), 384.0, np.float32)
        pc[:, :s.nchunk] = pcode.reshape(s.nchunk, 128).T
        pc = pc.astype(ml_dtypes.bfloat16)

        nodes0 = c * npc
        x_loc = x[nodes0:nodes0 + npc]
        rd_loc = recip_deg[nodes0:nodes0 + npc]

        xp = np.zeros((128, s.freep), np.float32)
        rdp = np.zeros((128, s.freep), np.float32)
        for w in range(n_win):
            bp = 32 * (w % PACK)
            c0 = (w // PACK) * WIN
            xp[bp:bp + 3, c0:c0 + WIN] = x_loc[w * WIN:(w + 1) * WIN].T
            rdp[bp:bp + 32, c0:c0 + WIN] = rd_loc[w * WIN:(w + 1) * WIN]

        pool_ind = np.zeros((npc, 256), ml_dtypes.bfloat16)
        bloc = np.asarray(batch[nodes0:nodes0 + npc], np.int64)
        pf = np.zeros((npc, 256), np.float32)
        pf[np.arange(npc), bloc] = recip_gcnt[bloc]
        pool_ind[:] = pf.astype(ml_dtypes.bfloat16)

        per_core.append(dict(
            idx16=idx16, pcode=pc, xp=xp, rdp=rdp,
            pool_ind=pool_ind, xtab=xtab,
        ))
    return s, per_core


def build_weight_inputs(s, w1l, b1l, w1r, bn1_g, bn1_b, w2l, b2l, w2r,
                        bn2_g, bn2_b, wlin, blin):
    PACK = s.pack

    def rep_rows(w, k):
        out = np.zeros((128, 32), np.float32)
        for g in range(PACK):
            out[32 * g:32 * g + k] = np.asarray(w, np.float32)
        return out

    col = lambda v: np.asarray(v, np.float32).reshape(-1, 1)
    iota = np.tile(np.arange(250, dtype=np.float32), 8).reshape(1, 2000)
    iota = np.repeat(iota, 128, 0).astype(ml_dtypes.bfloat16)
    return dict(
        W1l=rep_rows(w1l, 3), W1r=rep_rows(w1r, 3),
        W2l=rep_rows(w2l, 32), W2r=rep_rows(w2r, 32),
        fold=np.tile(np.eye(32, dtype=np.float32), (4, 1)),
        b1l=col(b1l), bn1_g=col(bn1_g), bn1_b=col(bn1_b),
        b2l=col(b2l), bn2_g=col(bn2_g), bn2_b=col(bn2_b),
        wlin=col(wlin),
        blin_rep=np.full((128, 1), float(np.asarray(blin).ravel()[0]),
                         np.float32),
        iden=np.eye(128, dtype=np.float32),
        idenb=np.eye(128, dtype=ml_dtypes.bfloat16),
        iota=iota,
    )


# ----------------------------------------------------------------------------
# Device program
# ----------------------------------------------------------------------------

def build_program(s):
    n_pack_rows = s.n_nodes // 4
    npc = s.nodes_per_core

    nc = bacc.Bacc("TRN2", target_bir_lowering=False, debug=False,
                   num_devices=N_CORES, num_swdge_queues=NQ)

    def din(name, shape, dt):
        return nc.dram_tensor(name, shape, dt, kind="ExternalInput").ap()

    T = dict(
        xtab=din("xtab", [n_pack_rows, 128], BF),
        idx16=din("idx16", [128, s.nslot // 16], I16),
        pcode=din("pcode", [128, s.nchunk8], BF),
        xp=din("xp", [128, s.freep], FP),
        rdp=din("rdp", [128, s.freep], FP),
        pool=din("pool_ind", [npc, 256], BF),
        W1l=din("W1l", [128, 32], FP),
        W1r=din("W1r", [128, 32], FP),
        W2l=din("W2l", [128, 32], FP),
        W2r=din("W2r", [128, 32], FP),
        fold=din("fold", [128, 32], FP),
        b1l=din("b1l", [32, 1], FP),
        bn1g=din("bn1_g", [32, 1], FP),
        bn1b=din("bn1_b", [32, 1], FP),
        b2l=din("b2l", [32, 1], FP),
        bn2g=din("bn2_g", [32, 1], FP),
        bn2b=din("bn2_b", [32, 1], FP),
        wlin=din("wlin", [32, 1], FP),
        blin=din("blin_rep", [128, 1], FP),
        iden=din("iden", [128, 128], FP),
        idenb=din("idenb", [128, 128], BF),
        iota=din("iota", [128, 2000], BF),
        out=nc.dram_tensor("out", [256], FP, kind="ExternalOutput").ap(),
        h1loc=nc.dram_tensor("h1loc", [npc * 32], BF).ap(),
        htab=nc.dram_tensor("htab", [n_pack_rows, 128], BF).ap(),
    )

    with tile.TileContext(nc) as tc:
        _body(tc, nc, s, T)
    nc.compile()
    return nc


def _body(tc, nc, s, T):
    npc = s.nodes_per_core
    n_win = s.n_win
    PACK = s.pack
    RG = [list(range(N_CORES))]
    AOT = mybir.AluOpType
    AFT = mybir.ActivationFunctionType

    stack = contextlib.ExitStack()
    pl = {}
    pl["P"] = stack.enter_context(tc.tile_pool(name="persist", bufs=1))
    pl["ring"] = stack.enter_context(tc.tile_pool(name="ring", bufs=16))
    pl["ind"] = stack.enter_context(tc.tile_pool(name="ind", bufs=5))
    pl["pcx"] = stack.enter_context(tc.tile_pool(name="pcx", bufs=2))
    pl["win"] = stack.enter_context(tc.tile_pool(name="win", bufs=3))
    pl["mw"] = stack.enter_context(tc.tile_pool(name="mw", bufs=3))
    pl["agg"] = stack.enter_context(tc.tile_pool(name="agg", bufs=4, space="PSUM"))
    pl["tp"] = stack.enter_context(tc.tile_pool(name="tp", bufs=3, space="PSUM"))
    pl["cp"] = stack.enter_context(tc.tile_pool(name="cp", bufs=1, space="PSUM"))
    pl["dram"] = stack.enter_context(tc.tile_pool(name="dram", bufs=1, space="DRAM"))
    pl["sm"] = stack.enter_context(tc.tile_pool(name="small", bufs=1))
    P = pl["P"]

    lib_inst = nc.gpsimd.load_library(mlp_lib)
    gst = {"count": 0, "prev": lib_inst}

    def load(name, shape, dt, src):
        t = P.tile(shape, dt, tag=name, name=name)
        nc.sync.dma_start(out=t[:], in_=src)
        return t

    # idx split: first gather call's slice loads alone so the layer-1 gather
    # stream starts without waiting for the full 6.5MB index table.
    nA = min(64, s.nslot // 16)
    idxA_sb = load("idxA", [128, nA], I16, T["idx16"][:, 0:nA])
    idxB_sb = load("idxB", [128, s.nslot // 16 - nA], I16, T["idx16"][:, nA:])

    def idx_slice(so, n):
        c0 = so // 16
        c1 = c0 + round_up_to_multiple(n, 16) // 16
        if c1 <= nA:
            return idxA_sb[:, c0:c1]
        assert c0 >= nA, (so, n)
        return idxB_sb[:, c0 - nA:c1 - nA]

    pcode_sb = load("pcodes", [128, s.nchunk8], BF, T["pcode"])
    iota_sb = load("iotas", [128, 2000], BF, T["iota"])
    rdp_sb = load("rdps", [128, s.freep], FP, T["rdp"])
    xp_sb = load("xps", [128, s.freep], FP, T["xp"])
    W1l_sb = load("W1ls", [128, 32], FP, T["W1l"])
    W1r_sb = load("W1rs", [128, 32], FP, T["W1r"])
    W2l_sb = load("W2ls", [128, 32], FP, T["W2l"])
    W2r_sb = load("W2rs", [128, 32], FP, T["W2r"])
    iden_sb = load("idens", [128, 128], FP, T["iden"])
    fold_sb = load("folds", [128, 32], FP, T["fold"])
    idenb_sb = load("idenbs", [128, 128], BF, T["idenb"])
    b1l_sb = load("b1ls", [32, 1], FP, T["b1l"])
    bn1g_sb = load("bn1gs", [32, 1], FP, T["bn1g"])
    bn1b_sb = load("bn1bs", [32, 1], FP, T["bn1b"])
    b2l_sb = load("b2ls", [32, 1], FP, T["b2l"])
    bn2g_sb = load("bn2gs", [32, 1], FP, T["bn2g"])
    bn2b_sb = load("bn2bs", [32, 1], FP, T["bn2b"])
    wlin_sb = load("wlins", [32, 1], FP, T["wlin"])
    blin_sb = load("blins", [128, 1], FP, T["blin"])

    aggsave = P.tile([128, s.freep], FP, tag="aggsave", name="aggsave")
    h1p = P.tile([128, s.freep], FP, tag="h1p", name="h1p")
    h1preTp = P.tile([128, s.freep], FP, tag="h1preTp", name="h1preTp")
    h2preTp = P.tile([128, s.freep], FP, tag="h2preTp", name="h2preTp")
    h2Tp = P.tile([128, s.freep], BF, tag="h2Tp", name="h2Tp")
    h1pack = P.tile([128, n_win * 32], BF, tag="h1pack", name="h1pack")

    def wpos(w):
        return (32 * (w % PACK),
                slice((w // PACK) * WIN, (w // PACK) * WIN + WIN))

    # ------------------------------------------------------------------
    def run_layer(layer, table_ap, Mrows):
        layer_state = {}
        # per-window BN stat slots: window w = PACK*k+g writes
        # [32g:32g+32, k] — filled by ACT accum_out on the preT copy (sum)
        # and a Square activation (sum of squares).
        msumb = pl["sm"].tile([128, n_win // PACK], FP,
                              tag=f"msumb{layer}", name="msumb")
        msqb = pl["sm"].tile([128, n_win // PACK], FP,
                             tag=f"msqb{layer}", name="msqb")
        win_psum = {}
        chunk_i = 0
        for (so, n, hh) in s.calls:
            ring_t = pl["ring"].tile([128, (CALL_MAX // 128) * 64], BF,
                                     tag="ring", name="ringt")
            gi = dma_gather_raw(
                nc.gpsimd,
                ring_t[:, 0:(round_up_to_multiple(n, 128) // 128) * 64]
                    .rearrange("p (s e) -> p s e", e=64),
                table_ap[:, 64 * hh:64 * hh + 64],
                idx_slice(so, n),
                n, 64, 128,
                queue_num=gst["count"] % NQ,
            )
            gi.ins.add_dependency(gst["prev"].ins.name,
                                  DependencyInfo.NO_SYNC_ONLY)
            gst["count"] += 1
            gst["prev"] = gi
            ncks = round_up_to_multiple(n, 128) // 128
            for ck in range(ncks):
                jj, hh2, first, last = s.chunk_meta[chunk_i]
                bp, wcol = wpos(jj)
                if chunk_i % 8 == 0:
                    ind8 = pl["ind"].tile([128, 2000], BF, tag="ind",
                                          name="ind8t")
                    g0 = chunk_i
                    if (chunk_i // 8) % 2 == 0:
                        # expand pcode on the (idle) scalar engine so the
                        # is_equal has all-packed operands → 2x DVE mode
                        pcx = pl["pcx"].tile([128, 2000], BF, tag="pcx",
                                             name="pcxt")
                        nc.scalar.activation(
                            pcx[:].rearrange("p (c o) -> p c o", o=250),
                            pcode_sb[:, g0:g0 + 8].to_broadcast([128, 8, 250]),
                            AFT.Copy)
                        nc.vector.tensor_tensor(
                            ind8[:], iota_sb[:, 0:2000], pcx[:], AOT.is_equal)
                    else:
                        nc.vector.tensor_tensor(
                            ind8[:].rearrange("p (c o) -> p c o", o=250),
                            iota_sb[:, 0:2000]
                                .rearrange("p (c o) -> p c o", o=250),
                            pcode_sb[:, g0:g0 + 8].to_broadcast([128, 8, 250]),
                            AOT.is_equal)
                    layer_state["ind8"] = ind8
                ind8 = layer_state["ind8"]
                co = (chunk_i % 8) * 250
                if jj not in win_psum:
                    win_psum[jj] = pl["agg"].tile(
                        [128, 128], FP, space="PSUM", tag="agg", name="aggt")
                ps = win_psum[jj]
                nc.tensor.matmul(
                    ps[bp:bp + Mrows, 0:125],
                    lhsT=ring_t[:, ck * 64: ck * 64 + Mrows],
                    rhs=ind8[:, co:co + 125],
                    start=first, stop=False,
                    tile_position=(0, bp))
                nc.tensor.matmul(
                    ps[bp:bp + Mrows, 0:125],
                    lhsT=ring_t[:, ck * 64 + 32: ck * 64 + 32 + Mrows],
                    rhs=ind8[:, co + 125:co + 250],
                    start=False, stop=last,
                    tile_position=(0, bp))
                chunk_i += 1
                if last and hh == 0:
                    # park the h=0 partial in SBUF; phase 1 adds it back
                    ps_t = win_psum.pop(jj)
                    nc.scalar.activation(aggsave[bp:bp + Mrows, wcol],
                                         ps_t[bp:bp + Mrows, 0:125], AFT.Copy)
                if last and hh == 1:
                    ps_t = win_psum.pop(jj)
                    meanw = pl["mw"].tile([128, 128], FP, tag="mw", name="mwt")
                    nc.vector.tensor_tensor(
                        meanw[bp:bp + Mrows, 0:125], ps_t[bp:bp + Mrows, 0:125],
                        aggsave[bp:bp + Mrows, wcol], AOT.add)
                    nc.vector.tensor_tensor(
                        meanw[bp:bp + Mrows, 0:125], meanw[bp:bp + Mrows, 0:125],
                        rdp_sb[bp:bp + Mrows, wcol], AOT.mult)
                    other = xp_sb if layer == 1 else h1p
                    ko = 3 if layer == 1 else 32
                    Wl = W1l_sb if layer == 1 else W2l_sb
                    Wr = W1r_sb if layer == 1 else W2r_sb
                    preT = h1preTp if layer == 1 else h2preTp
                    kcol = jj // PACK
                    pw = pl["tp"].tile([128, 128], FP, space="PSUM",
                                       tag="tp", name="pwt")
                    nc.tensor.matmul(
                        pw[bp:bp + 32, 0:125], lhsT=Wl[bp:bp + Mrows, :],
                        rhs=meanw[bp:bp + Mrows, 0:125], start=True, stop=False,
                        tile_position=(bp, bp))
                    nc.tensor.matmul(
                        pw[bp:bp + 32, 0:125], lhsT=Wr[bp:bp + ko, :],
                        rhs=other[bp:bp + ko, wcol], start=False, stop=True,
                        tile_position=(bp, bp))
                    nc.scalar.activation(preT[bp:bp + 32, wcol],
                                         pw[bp:bp + 32, 0:125], AFT.Copy,
                                         accum_out=msumb[bp:bp + 32,
                                                         kcol:kcol + 1])
                    sqd = pl["win"].tile([128, 128], FP, tag="sqd", name="sqd")
                    nc.scalar.activation(sqd[bp:bp + 32, 0:125],
                                         pw[bp:bp + 32, 0:125], AFT.Square,
                                         accum_out=msqb[bp:bp + 32,
                                                        kcol:kcol + 1])
        assert chunk_i == s.nchunk
        # reduce window slots and fold the 4 pack groups via one PE matmul
        statspair = pl["sm"].tile([128, 2], FP, tag=f"sp{layer}", name="spt")
        nc.vector.tensor_reduce(statspair[:, 0:1], msqb[:, :],
                                mybir.AxisListType.X, AOT.add)
        nc.vector.tensor_reduce(statspair[:, 1:2], msumb[:, :],
                                mybir.AxisListType.X, AOT.add)
        proj_ps = pl["tp"].tile([32, 2], FP, space="PSUM", tag="tp",
                                name="projps")
        nc.tensor.matmul(proj_ps[:, :], lhsT=fold_sb[:, :], rhs=statspair[:, :],
                         start=True, stop=True)
        proj = pl["sm"].tile([32, 2], FP, tag=f"proj{layer}", name="projt")
        nc.vector.tensor_copy(proj[:, :], proj_ps[:, :])
        return proj

    def allreduce_sb(src_sb_ap, shape, tag):
        dti = pl["dram"].tile(shape, FP, tag=f"ari{tag}", name="arit")
        dto = pl["dram"].tile(shape, FP, tag=f"aro{tag}", name="arot")
        nc.sync.dma_start(out=dti[:], in_=src_sb_ap)
        nc.gpsimd.collective_compute(
            "AllReduce", AOT.add, replica_groups=RG,
            ins=[dti.opt()], outs=[dto.opt()])
        res = pl["sm"].tile(shape, FP, tag=f"arr{tag}", name="arrt")
        nc.sync.dma_start(out=res[:], in_=dto[:])
        return res

    def bn_post(pg, bl_sb, g_sb, b_sb, tag):
        # var*N = pg0 - Ninv*pg1^2 ; std = sqrt(Ninv*(var*N) + EPS)
        SM = pl["sm"]
        Ninv = 1.0 / s.n_nodes
        t1 = SM.tile([32, 1], FP, tag=f"t1{tag}", name="t1s")
        nc.vector.tensor_tensor(t1[:], pg[:, 1:2], pg[:, 1:2], AOT.mult)
        nc.vector.tensor_scalar(t1[:], t1[:], -Ninv, None, AOT.mult)
        nc.vector.tensor_tensor(t1[:], t1[:], pg[:, 0:1], AOT.add)
        epsc = SM.tile([32, 1], FP, tag=f"eps{tag}", name="epsc")
        nc.vector.memset(epsc[:], EPS)
        sq = SM.tile([32, 1], FP, tag=f"sq{tag}", name="sqt")
        nc.scalar.activation(sq[:], t1[:], AFT.Sqrt, bias=epsc[:], scale=Ninv)
        y = SM.tile([32, 1], FP, tag=f"y{tag}", name="yt")
        nc.vector.reciprocal(y[:], sq[:])
        a = SM.tile([128, 1], FP, tag=f"a{tag}", name="at")
        cc = SM.tile([128, 1], FP, tag=f"c{tag}", name="cct")
        nc.vector.tensor_tensor(a[0:32], g_sb[:], y[:], AOT.mult)
        t = SM.tile([32, 1], FP, tag=f"t{tag}", name="tt")
        nc.vector.tensor_scalar(t[:], pg[:, 1:2], Ninv, None, AOT.mult)
        nc.vector.tensor_tensor(t[:], t[:], bl_sb[:], AOT.add)
        nc.vector.tensor_tensor(t[:], t[:], a[0:32], AOT.mult)
        nc.vector.tensor_tensor(cc[0:32], b_sb[:], t[:], AOT.subtract)
        for g in range(1, PACK):
            nc.sync.dma_start(out=a[32 * g:32 * g + 32], in_=a[0:32])
            nc.sync.dma_start(out=cc[32 * g:32 * g + 32], in_=cc[0:32])
        return a, cc

    # ================= LAYER 1 =================
    p1 = run_layer(1, T["xtab"], 3)
    p1g = allreduce_sb(p1[:], [32, 2], "c1")
    a1, c1 = bn_post(p1g, b1l_sb, bn1g_sb, bn1b_sb, "1")

    # finalize batched per pack-group: windows 4k..4k+3 fill all 128
    # partitions of column block k, and their h1pack column blocks are
    # contiguous, so one relu + one full-height transpose covers 4 windows.
    assert PACK == 4 and n_win % PACK == 0
    for k in range(n_win // PACK):
        wcol = slice(k * WIN, (k + 1) * WIN)
        nc.scalar.activation(h1p[:, wcol], h1preTp[:, wcol],
                             AFT.Relu, bias=c1[:], scale=a1[:])
    for k in range(n_win // PACK):
        wcol = slice(k * WIN, (k + 1) * WIN)
        tps = pl["tp"].tile([128, 128], FP, space="PSUM", tag="tp", name="h1tp")
        nc.tensor.transpose(tps[0:125, 0:128], h1p[:, wcol], iden_sb[:, :],
                            tile_position=(0, 0))
        nc.scalar.activation(h1pack[0:125, k * 128:(k + 1) * 128],
                             tps[0:125, 0:128], AFT.Copy)
    nc.sync.dma_start(
        out=T["h1loc"].rearrange("(w p f) -> p w f", p=WIN, f=32),
        in_=h1pack[0:125, 0:n_win * 32].rearrange("p (w f) -> p w f", f=32))
    nc.gpsimd.collective_compute(
        "AllGather", AOT.bypass, replica_groups=RG,
        ins=[T["h1loc"][:]], outs=[T["htab"].rearrange("a b -> (a b)")])

    # ================= LAYER 2 =================
    p2 = run_layer(2, T["htab"], 32)
    p2g = allreduce_sb(p2[:], [32, 2], "c2")
    a2, c2 = bn_post(p2g, b2l_sb, bn2g_sb, bn2b_sb, "2")

    pool_ps = pl["cp"].tile([32, 256], FP, space="PSUM", tag="C",
                            name="poolps")
    for k in range(n_win // PACK):
        wcol = slice(k * WIN, (k + 1) * WIN)
        nc.scalar.activation(h2Tp[:, wcol], h2preTp[:, wcol],
                             AFT.Relu, bias=c2[:], scale=a2[:])
        tps = pl["tp"].tile([128, 128], BF, space="PSUM", tag="tp", name="h2tp")
        nc.tensor.transpose(tps[0:125, 0:128], h2Tp[:, wcol], idenb_sb[:, :],
                            tile_position=(0, 0))
        h2w = pl["win"].tile([128, 128], BF, tag="h2w", name="h2w")
        nc.scalar.activation(h2w[0:125, :], tps[0:125, 0:128], AFT.Copy)
        pind4 = pl["win"].tile([128, PACK * 256], BF, tag="pind4", name="pind4")
        nc.sync.dma_start(
            out=pind4[0:125, :].rearrange("p (g c) -> p g c", c=256),
            in_=T["pool"][k * PACK * WIN:(k + 1) * PACK * WIN, :]
                .rearrange("(g n) c -> n g c", g=PACK))
        for g in range(PACK):
            w = PACK * k + g
            nc.tensor.matmul(pool_ps[:, :], lhsT=h2w[0:125, 32 * g:32 * g + 32],
                             rhs=pind4[0:125, 256 * g:256 * g + 256],
                             start=w == 0, stop=w == n_win - 1)

    gsum_sb = pl["sm"].tile([32, 256], FP, tag="gsum", name="gsum_sb")
    nc.vector.tensor_copy(gsum_sb[:], pool_ps[:, :])
    gT = allreduce_sb(gsum_sb[:], [32, 256], "pool")

    gw = pl["sm"].tile([32, 256], FP, tag="gw", name="gw")
    nc.vector.tensor_scalar(gw[:], gT[:], wlin_sb[:], None, AOT.mult)
    ones32 = pl["sm"].tile([32, 1], FP, tag="ones32", name="ones32")
    nc.vector.memset(ones32[:], 1.0)
    ostage = pl["sm"].tile([128, 2], FP, tag="ostage", name="ostage")
    for half in range(2):
        hp = pl["tp"].tile([128, 1], FP, space="PSUM", tag="tp", name="hpt")
        nc.tensor.matmul(hp[:, :], lhsT=gw[:, half * 128:(half + 1) * 128],
                         rhs=ones32[:, :], start=True, stop=True)
        nc.vector.tensor_tensor(ostage[:, half:half + 1], hp[:, :],
                                blin_sb[:], AOT.add)
    nc.sync.dma_start(out=T["out"][0:128], in_=ostage[:, 0:1])
    nc.sync.dma_start(out=T["out"][128:256], in_=ostage[:, 1:2])

    stack.close()


# ----------------------------------------------------------------------------
# Public entry point
# ----------------------------------------------------------------------------

N_NODES = 100000
N_GRAPHS = 256

LAST_EXEC_NS = None


def kernel(x, edge_index, batch, w1l, b1l, w1r, bn1_g, bn1_b,
           w2l, b2l, w2r, bn2_g, bn2_b, wlin, blin, _trace=False):
    global LAST_EXEC_NS
    from concourse.bass_utils import run_bass_kernel_spmd

    x = np.asarray(x, np.float32)
    edge_index = np.asarray(edge_index)
    batch = np.asarray(batch)

    s, per_core = build_schedule(x, edge_index, batch, N_NODES, N_GRAPHS,
                                 wgroup=4)
    wts = build_weight_inputs(s, w1l, b1l, w1r, bn1_g, bn1_b,
                              w2l, b2l, w2r, bn2_g, bn2_b, wlin, blin)
    nc = build_program(s)
    in_maps = []
    for c in range(N_CORES):
        m = dict(per_core[c])
        m.update(wts)
        in_maps.append(m)
    res = run_bass_kernel_spmd(nc, in_maps, list(range(N_CORES)),
                               trace=_trace)
    LAST_EXEC_NS = res.exec_time_ns
    return np.asarray(res.results[0]["out"], np.float32)

